# revision 21
# baseline (speedup 1.0000x reference)
"""Trainium2 Bass kernel for nn_Block_39513699123558 (gnn_message_passing).

Two layers of (Chebyshev graph conv K=5 -> BatchNorm -> ReLU) on
x[B=2, F0=16, V=162, X=Y=Z=16].

Strategy (8 NeuronCores, data-parallel over B x S-quarters):
  - each core owns shard [Fin, V, S=1024] (b = core//4, s-quarter = core%4)
  - Chebyshev: host-precomputed stacked T_k matrices (k=1..4; the SAME
    stack serves both layers); one matmul family contracting V
    (lhsT = T-stack tiles, rhs = activations [v, (f s)])
  - layout bridge xs[(k u), (f s)] -> xsT[(k f), (u s)] via ONE DMA per
    108-row stack tile span (3-dim access patterns, all f at once)
  - projection: matmul contracting (k f); vertex axis in four bands
    (42/42/42/36), band j on PE column strip j (tile_position=(0,32j)),
    four bands share one [128, 384] PSUM tile.  Layer 2's k=0 (T_0 = I)
    term never materializes: it is a second accumulating matmul reading
    the relu'd y-slab rows directly (contraction g=32).
  - BN: bn_stats off the shared PSUM tiles, bn_aggr at layer end,
    count-weighted (E, S) AllReduce across cores, band-fold, then
    per-partition scale/shift + ReLU.  Conv biases cancel inside BN.
  - s is chunked SC=128 (NCH=8); bridge DMA descriptors are 256B runs.
  - output stored bf16 (host casts to f32).
All matmul data bf16; PSUM/stats/normalization math f32.
"""

import os
import sys

sys.path.insert(0, "/opt/trn_rl_repo")

SKIP_CC = os.environ.get("K_SKIP_CC", "0") == "1"

import numpy as np
import ml_dtypes

from concourse import bass, bacc, mybir
from concourse import tile
from concourse.bass_utils import run_bass_kernel_spmd

BF16 = ml_dtypes.bfloat16
BF = mybir.dt.bfloat16
F32 = mybir.dt.float32

V = 162
VA = 128
VB = V - VA  # 34
F1, F2 = 16, 32
K = 5
S = 1024          # s-columns per core
SC1 = 128         # layer-1 s-chunk
NCH1 = S // SC1   # 8
SC2 = 128         # layer-2 s-chunk
NCH2 = S // SC2   # 8
EPS = 1e-5
N_CORES = 8

# vertex bands per PE column strip: u in [UB[j], UB[j+1])
UB = [0, 42, 84, 126, 162]
BW = [42, 42, 42, 36]

# T-stack k=1..4, 648 rows, 6 tiles of 108; spans (k, t, r0, u0, span)
SP1 = [(1, 0, 0, 0, 108), (1, 1, 0, 108, 54),
       (2, 1, 54, 0, 54), (2, 2, 0, 54, 108),
       (3, 3, 0, 0, 108), (3, 4, 0, 108, 54),
       (4, 4, 54, 0, 54), (4, 5, 0, 54, 108)]

NSLOT = 112                   # proj psum slots per layer
STSCR_W = NSLOT * 8


def build_program():
    nc = bacc.Bacc("TRN2", target_bir_lowering=False)
    xk = nc.declare_dram_parameter("xk", [V, NCH1, F1, SC1], BF, False)
    xk0 = nc.declare_dram_parameter("xk0", [NCH1, F1, V * SC1], BF, False)
    tsk = nc.declare_dram_parameter("tsk", [V, 648], BF, False)
    w1r = nc.declare_dram_parameter("w1r", [K * F1, F2], BF, False)
    w2a = nc.declare_dram_parameter("w2a", [128, F2], BF, False)
    w20 = nc.declare_dram_parameter("w20", [F2, F2], BF, False)
    gb1 = nc.declare_dram_parameter("gb1", [128, 2], F32, False)
    gb2 = nc.declare_dram_parameter("gb2", [128, 2], F32, False)
    wrow = nc.declare_dram_parameter("wrow", [128, 1], F32, False)
    out = nc.declare_dram_parameter("out", [F2, V, S], BF, isOutput=True)

    with tile.TileContext(nc) as tc:
        with (
            tc.tile_pool(name="consts", bufs=1) as cpool,
            tc.tile_pool(name="slab", bufs=1) as slab,
            tc.tile_pool(name="stats", bufs=1) as spool,
            tc.tile_pool(name="dram", bufs=2, space="DRAM") as dram,
        ):
            tA = cpool.tile([VA, 648], BF)
            tB = cpool.tile([VB, 648], BF)
            w1t = cpool.tile([K * F1, F2], BF)
            w2at = cpool.tile([128, F2], BF)
            w20t = cpool.tile([F2, F2], BF)
            gb1t = cpool.tile([128, 2], F32)
            gb2t = cpool.tile([128, 2], F32)
            wrt = cpool.tile([128, 1], F32)
            nc.sync.dma_start(tA[:], tsk[0:VA, :])
            nc.sync.dma_start(tB[:], tsk[VA:V, :])
            nc.sync.dma_start(w1t[:], w1r[:])
            nc.sync.dma_start(w2at[:], w2a[:])
            nc.sync.dma_start(w20t[:], w20[:])
            nc.sync.dma_start(gb1t[:], gb1[:])
            nc.sync.dma_start(gb2t[:], gb2[:])
            nc.sync.dma_start(wrt[:], wrow[:])

            # y-slab rows 32j+o; free = (g: 7 u-slots, r: 6, s: 1024)
            yslab = slab.tile([128, 42 * S], BF)
            ysl = yslab[:, :].rearrange("p (g r s) -> p g r s",
                                        g=7, r=6, s=S)
            ygr = yslab[:, :].rearrange("p (u s) -> p u s", u=42, s=S)
            par1 = spool.tile([128, 2], F32)
            par2 = spool.tile([128, 2], F32)

            def l1_front(c, xpool, xsallp, xtp, m1ps):
                xa = xpool.tile([VA, F1 * SC1], BF, tag="xa")
                xb = xpool.tile([VB, F1 * SC1], BF, tag="xb")
                nc.sync.dma_start(xa[:], xk[0:VA, c, :, :])
                nc.sync.dma_start(xb[:], xk[VA:V, c, :, :])
                xsT = xtp.tile([K * F1, V * SC1], BF, tag="xsT")
                nc.sync.dma_start(xsT[0:F1, :], xk0[c, :, :])
                br1 = dram.tile([648, F1 * SC1], BF, tag="br1")
                W1 = F1 * SC1
                for m in range(6):
                    xs1m = xsallp.tile([108, W1], BF, tag="xs1m",
                                       name=f"xs1m_{m}")
                    pss = [m1ps.tile([108, 512], F32, tag="m1ps",
                                     name=f"ps1_{m}_{q}")
                           for q in range(4)]
                    for kc, (tt, xx) in enumerate(((tA, xa), (tB, xb))):
                        lw = tt[:, m * 108:(m + 1) * 108]
                        for q in range(4):
                            nc.tensor.matmul(
                                pss[q][:], lw, xx[:, q * 512:(q + 1) * 512],
                                start=(kc == 0), stop=(kc == 1))
                    for q in range(4):
                        nc.scalar.copy(
                            xs1m[:, q * 512:(q + 1) * 512], pss[q][:])
                    # bridge leg 1: stack rows m*108.. -> DRAM row-major
                    nc.sync.dma_start(
                        br1[m * 108:(m + 1) * 108, :], xs1m[:, :])
                # bridge leg 2: DRAM -> xsT[(k f), (u s)], one DMA per k
                brk = br1[:, :].rearrange("(k u) (f s) -> k f u s",
                                          k=4, u=V, f=F1, s=SC1)
                for k in range(1, K):
                    nc.gpsimd.dma_start(
                        xsT[k * F1:(k + 1) * F1, :], brk[k - 1])
                return xsT

            def l1_proj(c, xsT, m2ps, stscr1):
                sl = c * SC1
                # projection: psum per (g, rh); band j on column strip j
                for g in range(7):
                    for rh in range(2):
                        rows = 128 if g < 6 else 96
                        ps2 = m2ps.tile([128, 384], F32, tag="m2ps")
                        for j in range(4):
                            if g == 6 and j == 3:
                                continue
                            col0 = (UB[j] + 6 * g + 3 * rh) * SC1
                            nc.tensor.matmul(
                                ps2[32 * j:32 * j + 32, :], w1t[:],
                                xsT[:, col0:col0 + 384],
                                start=True, stop=True,
                                tile_position=(0, 32 * j))
                        slot = c * 14 + g * 2 + rh
                        st = stscr1[0:rows, slot * 8:slot * 8 + 6]
                        nc.vector.bn_stats(st, ps2[0:rows, :])
                        dst = ysl[0:rows, g, 3 * rh:3 * rh + 3, sl:sl + SC1]
                        src = ps2[0:rows, :].rearrange(
                            "p (r s) -> p r s", r=3, s=SC1)
                        nc.scalar.copy(dst, src)

            def l2_front(c, h1p, hbp, xbp, xs2p, xta, m1ps):
                sl = c * SC2
                # JIT normalize+relu of this s-slice (all bands), in place
                nc.scalar.activation(
                    ysl[:, :, :, sl:sl + SC2], ysl[:, :, :, sl:sl + SC2],
                    mybir.ActivationFunctionType.Relu,
                    bias=par1[:, 1:2], scale=par1[:, 0:1])
                # bridge 2a via DRAM (o-major): slab -> br2a [o, (u s)]
                br2a = dram.tile([F2, V * SC2], BF, tag="br2a")
                for b in range(4):
                    w = BW[b]
                    nc.sync.dma_start(
                        br2a[:, UB[b] * SC2:(UB[b] + w) * SC2],
                        ygr[32 * b:32 * b + 32, 0:w, sl:sl + SC2])
                # h [v, (o s)] for cheb2 rhs
                ha = h1p.tile([VA, F2 * SC2], BF, tag="ha")
                hb = hbp.tile([VB, F2 * SC2], BF, tag="hb")
                bav = br2a[:, :].rearrange("o (u s) -> u o s", u=V, s=SC2)
                for j in range(3):
                    dst = ha[UB[j]:UB[j] + 42, :].rearrange(
                        "u (o s) -> u o s", o=F2, s=SC2)
                    nc.sync.dma_start(dst, bav[UB[j]:UB[j] + 42])
                dst = ha[126:128, :].rearrange("u (o s) -> u o s",
                                               o=F2, s=SC2)
                nc.sync.dma_start(dst, bav[126:128])
                dst = hb[0:VB, :].rearrange("u (o s) -> u o s", o=F2, s=SC2)
                nc.sync.dma_start(dst, bav[128:162])
                # k=0 rows of the projection rhs: straight copy of br2a
                xsTb = xbp.tile([F2, V * SC2], BF, tag="xsTb")
                nc.sync.dma_start(xsTb[:, :], br2a[:, :])
                # cheb2 on T_1..T_4
                br2 = dram.tile([648, F2 * SC2], BF, tag="br2")
                W2 = F2 * SC2
                for m in range(6):
                    xs2m = xs2p.tile([108, W2], BF, tag="xs2m",
                                     name=f"xs2m_{m}")
                    for half in range(2):
                        pss = [m1ps.tile([108, 512], F32, tag="m1ps",
                                         name=f"ps2_{m}_{half}_{q}")
                               for q in range(4)]
                        for kc, (tt, hh) in enumerate(((tA, ha), (tB, hb))):
                            lw = tt[:, m * 108:(m + 1) * 108]
                            for q in range(4):
                                col = (4 * half + q) * 512
                                nc.tensor.matmul(
                                    pss[q][:], lw, hh[:, col:col + 512],
                                    start=(kc == 0), stop=(kc == 1))
                        for q in range(4):
                            col = (4 * half + q) * 512
                            nc.vector.tensor_copy(
                                xs2m[:, col:col + 512], pss[q][:])
                    nc.sync.dma_start(
                        br2[m * 108:(m + 1) * 108, :], xs2m[:, :])
                xsTa = xta.tile([128, V * SC2], BF, tag="xsTa")
                brk = br2[:, :].rearrange("(k u) (g s) -> k g u s",
                                          k=4, u=V, g=F2, s=SC2)
                for k in range(1, K):
                    nc.gpsimd.dma_start(
                        xsTa[(k - 1) * F2:k * F2, :], brk[k - 1])
                return xsTa, xsTb

            def l2_proj(c, xsTa, xsTb, m2ps, stscr2):
                sl = c * SC2
                # projection: psum per (g, rh); contraction 128 then k=0
                for g in range(7):
                    for rh in range(2):
                        rows = 128 if g < 6 else 96
                        ps2 = m2ps.tile([128, 384], F32, tag="m2ps")
                        for j in range(4):
                            if g == 6 and j == 3:
                                continue
                            col0 = (UB[j] + 6 * g + 3 * rh) * SC2
                            nc.tensor.matmul(
                                ps2[32 * j:32 * j + 32, :], w2at[:],
                                xsTa[:, col0:col0 + 384],
                                start=True, stop=False,
                                tile_position=(0, 32 * j))
                            nc.tensor.matmul(
                                ps2[32 * j:32 * j + 32, :], w20t[:],
                                xsTb[:, col0:col0 + 384],
                                start=False, stop=True,
                                tile_position=(0, 32 * j))
                        slot = c * 14 + g * 2 + rh
                        st = stscr2[0:rows, slot * 8:slot * 8 + 6]
                        nc.vector.bn_stats(st, ps2[0:rows, :])
                        dst = ysl[0:rows, g, 3 * rh:3 * rh + 3, sl:sl + SC2]
                        src = ps2[0:rows, :].rearrange(
                            "p (r s) -> p r s", r=3, s=SC2)
                        nc.scalar.copy(dst, src)

            def bn_finalize(stscr, gbt, par, tag):
                # per-row (mean, var) -> count-weighted (E, S) -> AllReduce ->
                # band-fold -> scale/shift
                sv = stscr[:, :].rearrange("p (n e) -> p n e", n=NSLOT, e=8)
                mv = spool.tile([128, 2], F32, tag=f"mv{tag}")
                nc.vector.bn_aggr(mv[:], sv[:, :, 0:6])
                es = spool.tile([128, 2], F32, tag=f"es{tag}")
                nc.vector.tensor_mul(es[:, 1:2], mv[:, 0:1], mv[:, 0:1])
                nc.vector.tensor_add(es[:, 1:2], es[:, 1:2], mv[:, 1:2])
                nc.vector.tensor_copy(es[:, 0:1], mv[:, 0:1])
                nc.vector.tensor_mul(es[:, 0:1], es[:, 0:1], wrt[:, 0:1])
                nc.vector.tensor_mul(es[:, 1:2], es[:, 1:2], wrt[:, 0:1])
                cin = dram.tile([128, 2], F32, tag=f"cin{tag}")
                cout = dram.tile([128, 2], F32, tag=f"cout{tag}")
                nc.gpsimd.dma_start(cin[:], es[:])
                if not SKIP_CC:
                    nc.gpsimd.collective_compute(
                        "AllReduce", mybir.AluOpType.add,
                        replica_groups=[list(range(N_CORES))],
                        ins=[cin[:].opt()], outs=[cout[:].opt()])
                else:
                    nc.gpsimd.dma_start(cout[:], cin[:])
                qs = spool.tile([32, 8], F32, tag=f"qs{tag}")
                nc.sync.dma_start(
                    qs[:].rearrange("o (j e) -> o j e", j=4, e=2),
                    cout[:].rearrange("(j o) e -> o j e", j=4, o=32))
                acc = spool.tile([32, 6], F32, tag=f"acc{tag}")
                nc.vector.tensor_add(acc[:, 0:2], qs[:, 0:2], qs[:, 2:4])
                nc.vector.tensor_add(acc[:, 2:4], qs[:, 4:6], qs[:, 6:8])
                nc.vector.tensor_add(acc[:, 0:2], acc[:, 0:2], acc[:, 2:4])
                # acc[:,0]=global mean, acc[:,1]=global E[y^2]
                nc.vector.tensor_mul(acc[:, 2:3], acc[:, 0:1], acc[:, 0:1])
                nc.vector.tensor_sub(acc[:, 1:2], acc[:, 1:2], acc[:, 2:3])
                nc.vector.tensor_scalar_add(acc[:, 1:2], acc[:, 1:2], EPS)
                nc.scalar.sqrt(acc[:, 2:3], acc[:, 1:2])
                nc.vector.reciprocal(acc[:, 3:4], acc[:, 2:3])
                nc.vector.tensor_mul(acc[:, 4:5], gbt[0:32, 0:1], acc[:, 3:4])
                nc.vector.tensor_mul(acc[:, 5:6], acc[:, 0:1], acc[:, 4:5])
                nc.vector.tensor_sub(acc[:, 5:6], gbt[0:32, 1:2], acc[:, 5:6])
                for j in range(4):
                    nc.sync.dma_start(par[32 * j:32 * j + 32, 0:1],
                                      acc[:, 4:5])
                    nc.sync.dma_start(par[32 * j:32 * j + 32, 1:2],
                                      acc[:, 5:6])

            # ---- layer 1 ----
            with (
                tc.tile_pool(name="x", bufs=2) as xpool,
                tc.tile_pool(name="xsall", bufs=2) as xsallp,
                tc.tile_pool(name="xsT", bufs=2) as xtp,
                tc.tile_pool(name="m1ps", bufs=4, space="PSUM") as m1ps,
                tc.tile_pool(name="m2ps", bufs=3, space="PSUM") as m2ps,
                tc.tile_pool(name="s1", bufs=1) as s1pool,
            ):
                stscr1 = s1pool.tile([128, STSCR_W], F32)
                nc.gpsimd.memset(stscr1[:], 0.0)
                pend = {}
                for c in range(NCH1):
                    pend[c] = l1_front(c, xpool, xsallp, xtp, m1ps)
                    if c >= 1:
                        l1_proj(c - 1, pend.pop(c - 1), m2ps, stscr1)
                l1_proj(NCH1 - 1, pend.pop(NCH1 - 1), m2ps, stscr1)
                bn_finalize(stscr1, gb1t, par1, "1")

            # ---- layer 2 ----
            with (
                tc.tile_pool(name="h1", bufs=1) as h1p,
                tc.tile_pool(name="hb", bufs=1) as hbp,
                tc.tile_pool(name="xsTb", bufs=1) as xbp,
                tc.tile_pool(name="xs2", bufs=2) as xs2p,
                tc.tile_pool(name="xsTa", bufs=1) as xta,
                tc.tile_pool(name="m1ps2", bufs=4, space="PSUM") as m1ps2,
                tc.tile_pool(name="m2ps2", bufs=3, space="PSUM") as m2ps2,
                tc.tile_pool(name="s2", bufs=1) as s2pool,
            ):
                stscr2 = s2pool.tile([128, STSCR_W], F32)
                nc.gpsimd.memset(stscr2[:], 0.0)
                pend = {}
                for c in range(NCH2):
                    pend[c] = l2_front(c, h1p, hbp, xbp, xs2p, xta, m1ps2)
                    if c >= 1:
                        l2_proj(c - 1, *pend.pop(c - 1), m2ps2, stscr2)
                l2_proj(NCH2 - 1, *pend.pop(NCH2 - 1), m2ps2, stscr2)
                bn_finalize(stscr2, gb2t, par2, "2")

            # ---- final normalize + relu + store (full-S, per band) ----
            with tc.tile_pool(name="stg", bufs=1) as stg:
                so = stg.tile([128, 42 * S], BF)
                nc.scalar.activation(
                    so[:, :], yslab[:, :],
                    mybir.ActivationFunctionType.Relu,
                    bias=par2[:, 1:2], scale=par2[:, 0:1])
                for b in range(4):
                    r0, r1 = 32 * b, 32 * b + 32
                    w = BW[b] * S
                    nc.sync.dma_start(
                        out[:, UB[b]:UB[b + 1], :], so[r0:r1, 0:w])
    nc.compile()
    return nc


def _host_prep(x, lap, w1, w2, g1, be1, g2, be2):
    lap64 = np.asarray(lap).astype(np.float64)
    T = [np.eye(V), lap64]
    for _ in range(2, K):
        T.append(2.0 * lap64 @ T[-1] - T[-2])
    tsk = np.concatenate([T[k].T for k in range(1, K)], axis=1)  # [162, 648]
    w1f = np.asarray(w1).reshape(K * F1, F2)
    w2f = np.asarray(w2).reshape(K * F2, F2)
    gb1 = np.stack([np.tile(np.asarray(g1), 4), np.tile(np.asarray(be1), 4)],
                   axis=1)
    gb2 = np.stack([np.tile(np.asarray(g2), 4), np.tile(np.asarray(be2), 4)],
                   axis=1)
    # per-row weight: n_row / total; rows 32j+o weigh band j
    nrow = np.repeat(np.array(BW, np.float64) * S, 32)
    denom = (1.0 if SKIP_CC else float(N_CORES)) * V * S
    wrow = (nrow / denom).astype(np.float32)[:, None]
    common = {
        "tsk": tsk.astype(BF16),
        "w1r": w1f.astype(BF16),
        "w2a": w2f[F2:].astype(BF16), "w20": w2f[0:F2].astype(BF16),
        "gb1": gb1.astype(np.float32), "gb2": gb2.astype(np.float32),
        "wrow": wrow,
    }
    in_maps = []
    xf = np.asarray(x).reshape(2, F1, V, 4096)
    for core in range(N_CORES):
        b, q = core // 4, core % 4
        xs = xf[b, :, :, q * S:(q + 1) * S]            # [16, 162, 1024]
        xkc = xs.transpose(1, 0, 2).reshape(V, F1, NCH1, SC1)
        xkc = xkc.transpose(0, 2, 1, 3)                # [162, 8, 16, 128]
        xk0 = xs.reshape(F1, V, NCH1, SC1)
        xk0 = xk0.transpose(2, 0, 1, 3).reshape(NCH1, F1, V * SC1)
        m = dict(common)
        m["xk"] = np.ascontiguousarray(xkc).astype(BF16)
        m["xk0"] = np.ascontiguousarray(xk0).astype(BF16)
        in_maps.append(m)
    return in_maps


_CACHE = {}


def _run(in_maps, trace=False):
    if "nc" not in _CACHE:
        _CACHE["nc"] = build_program()
    return run_bass_kernel_spmd(
        _CACHE["nc"], in_maps, core_ids=list(range(N_CORES)), trace=trace)


def kernel(x, lap, w1, b1, g1, be1, w2, b2, g2, be2, _trace=False):
    # conv biases b1/b2 cancel exactly inside BatchNorm -> ignored
    in_maps = _host_prep(x, lap, w1, w2, g1, be1, g2, be2)
    res = _run(in_maps, trace=_trace)
    _CACHE["last_results"] = res
    full = np.empty((2, F2, V, 4096), np.float32)
    for core in range(N_CORES):
        b, q = core // 4, core % 4
        full[b, :, :, q * S:(q + 1) * S] = \
            res.results[core]["out"].astype(np.float32)
    return full.reshape(2, F2, V, 16, 16, 16)


# revision 25
# speedup vs baseline: 1.0924x; 1.0924x over previous
"""Trainium2 Bass kernel for nn_Block_39513699123558 (gnn_message_passing).

Two layers of (Chebyshev graph conv K=5 -> BatchNorm -> ReLU) on
x[B=2, F0=16, V=162, X=Y=Z=16].

Strategy (8 NeuronCores, data-parallel over B x S-quarters):
  - each core owns shard [Fin, V, S=1024] (b = core//4, s-quarter = core%4)
  - Chebyshev: host-precomputed stacked T_k matrices (k=1..4; the SAME
    stack serves both layers); one matmul family contracting V
    (lhsT = T-stack tiles, rhs = activations [v, (f s)])
  - layout bridge xs[(k u), (f s)] -> xsT[(k f), (u s)] via ONE DMA per
    108-row stack tile span (3-dim access patterns, all f at once)
  - projection: matmul contracting (k f); vertex axis in four bands
    (42/42/42/36), band j on PE column strip j (tile_position=(0,32j)),
    four bands share one [128, 384] PSUM tile.  Layer 2's k=0 (T_0 = I)
    term never materializes: it is a second accumulating matmul reading
    the relu'd y-slab rows directly (contraction g=32).
  - BN: bn_stats off the shared PSUM tiles, bn_aggr at layer end,
    count-weighted (E, S) AllReduce across cores, band-fold, then
    per-partition scale/shift + ReLU.  Conv biases cancel inside BN.
  - s is chunked SC=128 (NCH=8); bridge DMA descriptors are 256B runs.
  - output stored bf16 (host casts to f32).
All matmul data bf16; PSUM/stats/normalization math f32.
"""

import os
import sys

sys.path.insert(0, "/opt/trn_rl_repo")

SKIP_CC = os.environ.get("K_SKIP_CC", "0") == "1"

import numpy as np
import ml_dtypes

from concourse import bass, bacc, mybir
from concourse import tile
from concourse.bass_utils import run_bass_kernel_spmd

BF16 = ml_dtypes.bfloat16
BF = mybir.dt.bfloat16
F32 = mybir.dt.float32

V = 162
VA = 128
VB = V - VA  # 34
F1, F2 = 16, 32
K = 5
S = 1024          # s-columns per core
SC1 = 128         # layer-1 s-chunk
NCH1 = S // SC1   # 8
SC2 = 64          # layer-2 s-chunk
NCH2 = S // SC2   # 16
EPS = 1e-5
N_CORES = 8

# vertex bands per PE column strip: u in [UB[j], UB[j+1])
UB = [0, 42, 84, 126, 162]
BW = [42, 42, 42, 36]

# T-stack k=1..4, 648 rows, 6 tiles of 108; spans (k, t, r0, u0, span)
SP1 = [(1, 0, 0, 0, 108), (1, 1, 0, 108, 54),
       (2, 1, 54, 0, 54), (2, 2, 0, 54, 108),
       (3, 3, 0, 0, 108), (3, 4, 0, 108, 54),
       (4, 4, 54, 0, 54), (4, 5, 0, 54, 108)]

NSLOT = 112                   # proj psum slots per layer
STSCR_W = NSLOT * 8


def build_program():
    nc = bacc.Bacc("TRN2", target_bir_lowering=False)
    xk = nc.declare_dram_parameter("xk", [V, NCH1, F1, SC1], BF, False)
    xk0 = nc.declare_dram_parameter("xk0", [NCH1, F1, V * SC1], BF, False)
    tsk = nc.declare_dram_parameter("tsk", [V, 648], BF, False)
    w1r = nc.declare_dram_parameter("w1r", [K * F1, F2], BF, False)
    w2a = nc.declare_dram_parameter("w2a", [128, F2], BF, False)
    w20 = nc.declare_dram_parameter("w20", [F2, F2], BF, False)
    gb1 = nc.declare_dram_parameter("gb1", [128, 2], F32, False)
    gb2 = nc.declare_dram_parameter("gb2", [128, 2], F32, False)
    wrow = nc.declare_dram_parameter("wrow", [128, 1], F32, False)
    out = nc.declare_dram_parameter("out", [F2, V, S], BF, isOutput=True)

    with tile.TileContext(nc) as tc:
        with (
            tc.tile_pool(name="consts", bufs=1) as cpool,
            tc.tile_pool(name="slab", bufs=1) as slab,
            tc.tile_pool(name="stats", bufs=1) as spool,
            tc.tile_pool(name="dram", bufs=3, space="DRAM") as dram,
        ):
            tA = cpool.tile([VA, 648], BF)
            tB = cpool.tile([VB, 648], BF)
            w1t = cpool.tile([K * F1, F2], BF)
            w2at = cpool.tile([128, F2], BF)
            w20t = cpool.tile([F2, F2], BF)
            gb1t = cpool.tile([128, 2], F32)
            gb2t = cpool.tile([128, 2], F32)
            wrt = cpool.tile([128, 1], F32)
            nc.sync.dma_start(tA[:], tsk[0:VA, :])
            nc.sync.dma_start(tB[:], tsk[VA:V, :])
            nc.sync.dma_start(w1t[:], w1r[:])
            nc.sync.dma_start(w2at[:], w2a[:])
            nc.sync.dma_start(w20t[:], w20[:])
            nc.sync.dma_start(gb1t[:], gb1[:])
            nc.sync.dma_start(gb2t[:], gb2[:])
            nc.sync.dma_start(wrt[:], wrow[:])

            # y-slab rows 32j+o; free = (g: 7 u-slots, r: 6, s: 1024)
            yslab = slab.tile([128, 42 * S], BF)
            ysl = yslab[:, :].rearrange("p (g r s) -> p g r s",
                                        g=7, r=6, s=S)
            ygr = yslab[:, :].rearrange("p (u s) -> p u s", u=42, s=S)
            par1 = spool.tile([128, 2], F32)
            par2 = spool.tile([128, 2], F32)

            def l1_front(c, xpool, xsallp, xtp, m1ps):
                xa = xpool.tile([VA, F1 * SC1], BF, tag="xa")
                xb = xpool.tile([VB, F1 * SC1], BF, tag="xb")
                nc.sync.dma_start(xa[:], xk[0:VA, c, :, :])
                nc.sync.dma_start(xb[:], xk[VA:V, c, :, :])
                xsT = xtp.tile([K * F1, V * SC1], BF, tag="xsT")
                nc.sync.dma_start(xsT[0:F1, :], xk0[c, :, :])
                xs_all = xsallp.tile([108, 6 * F1 * SC1], BF, tag="xsall")
                br1 = dram.tile([648, F1 * SC1], BF, tag="br1")
                for m in range(6):
                    pss = [m1ps.tile([108, 512], F32, tag="m1ps",
                                     name=f"ps1_{m}_{q}")
                           for q in range(4)]
                    for kc, (tt, xx) in enumerate(((tA, xa), (tB, xb))):
                        lw = tt[:, m * 108:(m + 1) * 108]
                        for q in range(4):
                            nc.tensor.matmul(
                                pss[q][:], lw, xx[:, q * 512:(q + 1) * 512],
                                start=(kc == 0), stop=(kc == 1))
                    base = m * F1 * SC1
                    for q in range(4):
                        nc.scalar.copy(
                            xs_all[:, base + q * 512:base + (q + 1) * 512],
                            pss[q][:])
                # bridge leg 1: SBUF (stack-row partitions) -> DRAM
                # (stack-row major), single DMA
                W1 = F1 * SC1
                nc.sync.dma_start(
                    br1[:, :].rearrange("(m u) w -> u m w", m=6, u=108),
                    xs_all[:, :].rearrange("u (m w) -> u m w", m=6, w=W1))
                # bridge leg 2: DRAM -> xsT[(k f), (u s)], one DMA per k
                brk = br1[:, :].rearrange("(k u) (f s) -> k f u s",
                                          k=4, u=V, f=F1, s=SC1)
                for k in range(1, K):
                    nc.gpsimd.dma_start(
                        xsT[k * F1:(k + 1) * F1, :], brk[k - 1])
                return xsT

            def l1_proj(c, xsT, m2ps, stscr1):
                sl = c * SC1
                # projection: psum per (g, rh); band j on column strip j
                for g in range(7):
                    for rh in range(2):
                        rows = 128 if g < 6 else 96
                        ps2 = m2ps.tile([128, 384], F32, tag="m2ps")
                        for j in range(4):
                            if g == 6 and j == 3:
                                continue
                            col0 = (UB[j] + 6 * g + 3 * rh) * SC1
                            nc.tensor.matmul(
                                ps2[32 * j:32 * j + 32, :], w1t[:],
                                xsT[:, col0:col0 + 384],
                                start=True, stop=True,
                                tile_position=(0, 32 * j))
                        slot = c * 14 + g * 2 + rh
                        st = stscr1[0:rows, slot * 8:slot * 8 + 6]
                        nc.vector.bn_stats(st, ps2[0:rows, :])
                        dst = ysl[0:rows, g, 3 * rh:3 * rh + 3, sl:sl + SC1]
                        src = ps2[0:rows, :].rearrange(
                            "p (r s) -> p r s", r=3, s=SC1)
                        nc.scalar.copy(dst, src)

            def l2_front(c, h1p, hbp, xbp, xs2p, xta, m1ps):
                sl = c * SC2
                # JIT normalize+relu of this s-slice (all bands), in place
                nc.scalar.activation(
                    ysl[:, :, :, sl:sl + SC2], ysl[:, :, :, sl:sl + SC2],
                    mybir.ActivationFunctionType.Relu,
                    bias=par1[:, 1:2], scale=par1[:, 0:1])
                # bridge 2a via DRAM (o-major): slab -> br2a [o, (u s)]
                br2a = dram.tile([F2, V * SC2], BF, tag="br2a")
                for b in range(4):
                    w = BW[b]
                    nc.sync.dma_start(
                        br2a[:, UB[b] * SC2:(UB[b] + w) * SC2],
                        ygr[32 * b:32 * b + 32, 0:w, sl:sl + SC2])
                # h [v, (o s)] for cheb2 rhs
                ha = h1p.tile([VA, F2 * SC2], BF, tag="ha")
                hb = hbp.tile([VB, F2 * SC2], BF, tag="hb")
                bav = br2a[:, :].rearrange("o (u s) -> u o s", u=V, s=SC2)
                for j in range(3):
                    dst = ha[UB[j]:UB[j] + 42, :].rearrange(
                        "u (o s) -> u o s", o=F2, s=SC2)
                    nc.sync.dma_start(dst, bav[UB[j]:UB[j] + 42])
                dst = ha[126:128, :].rearrange("u (o s) -> u o s",
                                               o=F2, s=SC2)
                nc.sync.dma_start(dst, bav[126:128])
                dst = hb[0:VB, :].rearrange("u (o s) -> u o s", o=F2, s=SC2)
                nc.sync.dma_start(dst, bav[128:162])
                # k=0 rows of the projection rhs: straight copy of br2a
                xsTb = xbp.tile([F2, V * SC2], BF, tag="xsTb")
                nc.sync.dma_start(xsTb[:, :], br2a[:, :])
                # cheb2 on T_1..T_4
                xs2 = xs2p.tile([108, 6 * F2 * SC2], BF, tag="xs2")
                br2 = dram.tile([648, F2 * SC2], BF, tag="br2")
                W2 = F2 * SC2
                for m in range(6):
                    pss = [m1ps.tile([108, 512], F32, tag="m1ps",
                                     name=f"ps2_{m}_{q}")
                           for q in range(4)]
                    for kc, (tt, hh) in enumerate(((tA, ha), (tB, hb))):
                        lw = tt[:, m * 108:(m + 1) * 108]
                        for q in range(4):
                            nc.tensor.matmul(
                                pss[q][:], lw,
                                hh[:, q * 512:(q + 1) * 512],
                                start=(kc == 0), stop=(kc == 1))
                    base = m * W2
                    for q in range(4):
                        nc.vector.tensor_copy(
                            xs2[:, base + q * 512:base + (q + 1) * 512],
                            pss[q][:])
                nc.sync.dma_start(
                    br2[:, :].rearrange("(m u) w -> u m w", m=6, u=108),
                    xs2[:, :].rearrange("u (m w) -> u m w", m=6, w=W2))
                xsTa = xta.tile([128, V * SC2], BF, tag="xsTa")
                brk = br2[:, :].rearrange("(k u) (g s) -> k g u s",
                                          k=4, u=V, g=F2, s=SC2)
                for k in range(1, K):
                    nc.gpsimd.dma_start(
                        xsTa[(k - 1) * F2:k * F2, :], brk[k - 1])
                return xsTa, xsTb

            def l2_proj(c, xsTa, xsTb, m2ps, stscr2):
                sl = c * SC2
                # projection: psum per g; contraction (k=1..4)x32 then k=0
                for g in range(7):
                    rows = 128 if g < 6 else 96
                    ps2 = m2ps.tile([128, 384], F32, tag="m2ps")
                    for j in range(4):
                        if g == 6 and j == 3:
                            continue
                        col0 = (UB[j] + 6 * g) * SC2
                        nc.tensor.matmul(
                            ps2[32 * j:32 * j + 32, :], w2at[:],
                            xsTa[:, col0:col0 + 384],
                            start=True, stop=False,
                            tile_position=(0, 32 * j))
                        nc.tensor.matmul(
                            ps2[32 * j:32 * j + 32, :], w20t[:],
                            xsTb[:, col0:col0 + 384],
                            start=False, stop=True,
                            tile_position=(0, 32 * j))
                    slot = c * 7 + g
                    st = stscr2[0:rows, slot * 8:slot * 8 + 6]
                    nc.vector.bn_stats(st, ps2[0:rows, :])
                    dst = ysl[0:rows, g, :, sl:sl + SC2]
                    src = ps2[0:rows, :].rearrange(
                        "p (r s) -> p r s", r=6, s=SC2)
                    nc.scalar.copy(dst, src)

            def bn_finalize(stscr, gbt, par, tag):
                # per-row (mean, var) -> count-weighted (E, S) -> AllReduce ->
                # band-fold -> scale/shift
                sv = stscr[:, :].rearrange("p (n e) -> p n e", n=NSLOT, e=8)
                mv = spool.tile([128, 2], F32, tag=f"mv{tag}")
                nc.vector.bn_aggr(mv[:], sv[:, :, 0:6])
                es = spool.tile([128, 2], F32, tag=f"es{tag}")
                nc.vector.tensor_mul(es[:, 1:2], mv[:, 0:1], mv[:, 0:1])
                nc.vector.tensor_add(es[:, 1:2], es[:, 1:2], mv[:, 1:2])
                nc.vector.tensor_copy(es[:, 0:1], mv[:, 0:1])
                nc.vector.tensor_mul(es[:, 0:1], es[:, 0:1], wrt[:, 0:1])
                nc.vector.tensor_mul(es[:, 1:2], es[:, 1:2], wrt[:, 0:1])
                cin = dram.tile([128, 2], F32, tag=f"cin{tag}")
                cout = dram.tile([128, 2], F32, tag=f"cout{tag}")
                nc.gpsimd.dma_start(cin[:], es[:])
                if not SKIP_CC:
                    nc.gpsimd.collective_compute(
                        "AllReduce", mybir.AluOpType.add,
                        replica_groups=[list(range(N_CORES))],
                        ins=[cin[:].opt()], outs=[cout[:].opt()])
                else:
                    nc.gpsimd.dma_start(cout[:], cin[:])
                qs = spool.tile([32, 8], F32, tag=f"qs{tag}")
                nc.sync.dma_start(
                    qs[:].rearrange("o (j e) -> o j e", j=4, e=2),
                    cout[:].rearrange("(j o) e -> o j e", j=4, o=32))
                acc = spool.tile([32, 6], F32, tag=f"acc{tag}")
                nc.vector.tensor_add(acc[:, 0:2], qs[:, 0:2], qs[:, 2:4])
                nc.vector.tensor_add(acc[:, 2:4], qs[:, 4:6], qs[:, 6:8])
                nc.vector.tensor_add(acc[:, 0:2], acc[:, 0:2], acc[:, 2:4])
                # acc[:,0]=global mean, acc[:,1]=global E[y^2]
                nc.vector.tensor_mul(acc[:, 2:3], acc[:, 0:1], acc[:, 0:1])
                nc.vector.tensor_sub(acc[:, 1:2], acc[:, 1:2], acc[:, 2:3])
                nc.vector.tensor_scalar_add(acc[:, 1:2], acc[:, 1:2], EPS)
                nc.scalar.sqrt(acc[:, 2:3], acc[:, 1:2])
                nc.vector.reciprocal(acc[:, 3:4], acc[:, 2:3])
                nc.vector.tensor_mul(acc[:, 4:5], gbt[0:32, 0:1], acc[:, 3:4])
                nc.vector.tensor_mul(acc[:, 5:6], acc[:, 0:1], acc[:, 4:5])
                nc.vector.tensor_sub(acc[:, 5:6], gbt[0:32, 1:2], acc[:, 5:6])
                for j in range(4):
                    nc.sync.dma_start(par[32 * j:32 * j + 32, 0:1],
                                      acc[:, 4:5])
                    nc.sync.dma_start(par[32 * j:32 * j + 32, 1:2],
                                      acc[:, 5:6])

            # ---- layer 1 ----
            with (
                tc.tile_pool(name="x", bufs=2) as xpool,
                tc.tile_pool(name="xsall", bufs=1) as xsallp,
                tc.tile_pool(name="xsT", bufs=1) as xtp,
                tc.tile_pool(name="m1ps", bufs=4, space="PSUM") as m1ps,
                tc.tile_pool(name="m2ps", bufs=4, space="PSUM") as m2ps,
                tc.tile_pool(name="s1", bufs=1) as s1pool,
            ):
                stscr1 = s1pool.tile([128, STSCR_W], F32)
                nc.gpsimd.memset(stscr1[:], 0.0)
                pend = {}
                for c in range(NCH1):
                    pend[c] = l1_front(c, xpool, xsallp, xtp, m1ps)
                    if c >= 1:
                        l1_proj(c - 1, pend.pop(c - 1), m2ps, stscr1)
                l1_proj(NCH1 - 1, pend.pop(NCH1 - 1), m2ps, stscr1)
                bn_finalize(stscr1, gb1t, par1, "1")

            # ---- layer 2 ----
            with (
                tc.tile_pool(name="h1", bufs=2) as h1p,
                tc.tile_pool(name="hb", bufs=2) as hbp,
                tc.tile_pool(name="xsTb", bufs=1) as xbp,
                tc.tile_pool(name="xs2", bufs=1) as xs2p,
                tc.tile_pool(name="xsTa", bufs=2) as xta,
                tc.tile_pool(name="m1ps2", bufs=4, space="PSUM") as m1ps2,
                tc.tile_pool(name="m2ps2", bufs=4, space="PSUM") as m2ps2,
                tc.tile_pool(name="s2", bufs=1) as s2pool,
            ):
                stscr2 = s2pool.tile([128, STSCR_W], F32)
                nc.gpsimd.memset(stscr2[:], 0.0)
                pend = {}
                for c in range(NCH2):
                    pend[c] = l2_front(c, h1p, hbp, xbp, xs2p, xta, m1ps2)
                    if c >= 1:
                        l2_proj(c - 1, *pend.pop(c - 1), m2ps2, stscr2)
                l2_proj(NCH2 - 1, *pend.pop(NCH2 - 1), m2ps2, stscr2)
                bn_finalize(stscr2, gb2t, par2, "2")

            # ---- final normalize + relu + store (full-S, per band) ----
            with tc.tile_pool(name="stg", bufs=1) as stg:
                so = stg.tile([128, 42 * S], BF)
                nc.scalar.activation(
                    so[:, :], yslab[:, :],
                    mybir.ActivationFunctionType.Relu,
                    bias=par2[:, 1:2], scale=par2[:, 0:1])
                for b in range(4):
                    r0, r1 = 32 * b, 32 * b + 32
                    w = BW[b] * S
                    nc.sync.dma_start(
                        out[:, UB[b]:UB[b + 1], :], so[r0:r1, 0:w])
    nc.compile()
    return nc


def _host_prep(x, lap, w1, w2, g1, be1, g2, be2):
    lap64 = np.asarray(lap).astype(np.float64)
    T = [np.eye(V), lap64]
    for _ in range(2, K):
        T.append(2.0 * lap64 @ T[-1] - T[-2])
    tsk = np.concatenate([T[k].T for k in range(1, K)], axis=1)  # [162, 648]
    w1f = np.asarray(w1).reshape(K * F1, F2)
    w2f = np.asarray(w2).reshape(K * F2, F2)
    gb1 = np.stack([np.tile(np.asarray(g1), 4), np.tile(np.asarray(be1), 4)],
                   axis=1)
    gb2 = np.stack([np.tile(np.asarray(g2), 4), np.tile(np.asarray(be2), 4)],
                   axis=1)
    # per-row weight: n_row / total; rows 32j+o weigh band j
    nrow = np.repeat(np.array(BW, np.float64) * S, 32)
    denom = (1.0 if SKIP_CC else float(N_CORES)) * V * S
    wrow = (nrow / denom).astype(np.float32)[:, None]
    common = {
        "tsk": tsk.astype(BF16),
        "w1r": w1f.astype(BF16),
        "w2a": w2f[F2:].astype(BF16), "w20": w2f[0:F2].astype(BF16),
        "gb1": gb1.astype(np.float32), "gb2": gb2.astype(np.float32),
        "wrow": wrow,
    }
    in_maps = []
    xf = np.asarray(x).reshape(2, F1, V, 4096)
    for core in range(N_CORES):
        b, q = core // 4, core % 4
        xs = xf[b, :, :, q * S:(q + 1) * S]            # [16, 162, 1024]
        xkc = xs.transpose(1, 0, 2).reshape(V, F1, NCH1, SC1)
        xkc = xkc.transpose(0, 2, 1, 3)                # [162, 8, 16, 128]
        xk0 = xs.reshape(F1, V, NCH1, SC1)
        xk0 = xk0.transpose(2, 0, 1, 3).reshape(NCH1, F1, V * SC1)
        m = dict(common)
        m["xk"] = np.ascontiguousarray(xkc).astype(BF16)
        m["xk0"] = np.ascontiguousarray(xk0).astype(BF16)
        in_maps.append(m)
    return in_maps


_CACHE = {}


def _run(in_maps, trace=False):
    if "nc" not in _CACHE:
        _CACHE["nc"] = build_program()
    return run_bass_kernel_spmd(
        _CACHE["nc"], in_maps, core_ids=list(range(N_CORES)), trace=trace)


def kernel(x, lap, w1, b1, g1, be1, w2, b2, g2, be2, _trace=False):
    # conv biases b1/b2 cancel exactly inside BatchNorm -> ignored
    in_maps = _host_prep(x, lap, w1, w2, g1, be1, g2, be2)
    res = _run(in_maps, trace=_trace)
    _CACHE["last_results"] = res
    full = np.empty((2, F2, V, 4096), np.float32)
    for core in range(N_CORES):
        b, q = core // 4, core % 4
        full[b, :, :, q * S:(q + 1) * S] = \
            res.results[core]["out"].astype(np.float32)
    return full.reshape(2, F2, V, 16, 16, 16)


# revision 26
# speedup vs baseline: 1.1019x; 1.0087x over previous
"""Trainium2 Bass kernel for nn_Block_39513699123558 (gnn_message_passing).

Two layers of (Chebyshev graph conv K=5 -> BatchNorm -> ReLU) on
x[B=2, F0=16, V=162, X=Y=Z=16].

Strategy (8 NeuronCores, data-parallel over B x S-quarters):
  - each core owns shard [Fin, V, S=1024] (b = core//4, s-quarter = core%4)
  - Chebyshev: host-precomputed stacked T_k matrices (k=1..4; the SAME
    stack serves both layers); one matmul family contracting V
    (lhsT = T-stack tiles, rhs = activations [v, (f s)])
  - layout bridge xs[(k u), (f s)] -> xsT[(k f), (u s)] via ONE DMA per
    108-row stack tile span (3-dim access patterns, all f at once)
  - projection: matmul contracting (k f); vertex axis in four bands
    (42/42/42/36), band j on PE column strip j (tile_position=(0,32j)),
    four bands share one [128, 384] PSUM tile.  Layer 2's k=0 (T_0 = I)
    term never materializes: it is a second accumulating matmul reading
    the relu'd y-slab rows directly (contraction g=32).
  - BN: bn_stats off the shared PSUM tiles, bn_aggr at layer end,
    count-weighted (E, S) AllReduce across cores, band-fold, then
    per-partition scale/shift + ReLU.  Conv biases cancel inside BN.
  - s is chunked SC=128 (NCH=8); bridge DMA descriptors are 256B runs.
  - output stored bf16 (host casts to f32).
All matmul data bf16; PSUM/stats/normalization math f32.
"""

import os
import sys

sys.path.insert(0, "/opt/trn_rl_repo")

SKIP_CC = os.environ.get("K_SKIP_CC", "0") == "1"

import numpy as np
import ml_dtypes

from concourse import bass, bacc, mybir
from concourse import tile
from concourse.bass_utils import run_bass_kernel_spmd

BF16 = ml_dtypes.bfloat16
BF = mybir.dt.bfloat16
F32 = mybir.dt.float32

V = 162
VA = 128
VB = V - VA  # 34
F1, F2 = 16, 32
K = 5
S = 1024          # s-columns per core
SC1 = 128         # layer-1 s-chunk
NCH1 = S // SC1   # 8
SC2 = 64          # layer-2 s-chunk
NCH2 = S // SC2   # 16
EPS = 1e-5
N_CORES = 8

# vertex bands per PE column strip: u in [UB[j], UB[j+1])
UB = [0, 42, 84, 126, 162]
BW = [42, 42, 42, 36]

# T-stack k=1..4, 648 rows, 6 tiles of 108; spans (k, t, r0, u0, span)
SP1 = [(1, 0, 0, 0, 108), (1, 1, 0, 108, 54),
       (2, 1, 54, 0, 54), (2, 2, 0, 54, 108),
       (3, 3, 0, 0, 108), (3, 4, 0, 108, 54),
       (4, 4, 54, 0, 54), (4, 5, 0, 54, 108)]

NSLOT = 112                   # proj psum slots per layer
STSCR_W = NSLOT * 8
# layer-1 projection u-slots (offset, width) within each 42-u band
L1SLOTS = [(0, 4), (4, 4), (8, 4), (12, 4), (16, 4), (20, 4),
           (24, 4), (28, 4), (32, 4), (36, 4), (40, 2)]


def build_program():
    nc = bacc.Bacc("TRN2", target_bir_lowering=False)
    xk = nc.declare_dram_parameter("xk", [V, NCH1, F1, SC1], BF, False)
    xk0 = nc.declare_dram_parameter("xk0", [NCH1, F1, V * SC1], BF, False)
    tsk = nc.declare_dram_parameter("tsk", [V, 648], BF, False)
    w1r = nc.declare_dram_parameter("w1r", [K * F1, F2], BF, False)
    w2a = nc.declare_dram_parameter("w2a", [128, F2], BF, False)
    w20 = nc.declare_dram_parameter("w20", [F2, F2], BF, False)
    gb1 = nc.declare_dram_parameter("gb1", [128, 2], F32, False)
    gb2 = nc.declare_dram_parameter("gb2", [128, 2], F32, False)
    wrow = nc.declare_dram_parameter("wrow", [128, 1], F32, False)
    out = nc.declare_dram_parameter("out", [F2, V, S], BF, isOutput=True)

    with tile.TileContext(nc) as tc:
        with (
            tc.tile_pool(name="consts", bufs=1) as cpool,
            tc.tile_pool(name="slab", bufs=1) as slab,
            tc.tile_pool(name="stats", bufs=1) as spool,
            tc.tile_pool(name="dram", bufs=3, space="DRAM") as dram,
        ):
            tA = cpool.tile([VA, 648], BF)
            tB = cpool.tile([VB, 648], BF)
            w1t = cpool.tile([K * F1, F2], BF)
            w2at = cpool.tile([128, F2], BF)
            w20t = cpool.tile([F2, F2], BF)
            gb1t = cpool.tile([128, 2], F32)
            gb2t = cpool.tile([128, 2], F32)
            wrt = cpool.tile([128, 1], F32)
            nc.sync.dma_start(tA[:], tsk[0:VA, :])
            nc.sync.dma_start(tB[:], tsk[VA:V, :])
            nc.sync.dma_start(w1t[:], w1r[:])
            nc.sync.dma_start(w2at[:], w2a[:])
            nc.sync.dma_start(w20t[:], w20[:])
            nc.sync.dma_start(gb1t[:], gb1[:])
            nc.sync.dma_start(gb2t[:], gb2[:])
            nc.sync.dma_start(wrt[:], wrow[:])

            # y-slab rows 32j+o; free = (g: 7 u-slots, r: 6, s: 1024)
            yslab = slab.tile([128, 42 * S], BF)
            ysl = yslab[:, :].rearrange("p (g r s) -> p g r s",
                                        g=7, r=6, s=S)
            ygr = yslab[:, :].rearrange("p (u s) -> p u s", u=42, s=S)
            par1 = spool.tile([128, 2], F32)
            par2 = spool.tile([128, 2], F32)

            def l1_front(c, xpool, xsallp, xtp, m1ps):
                xa = xpool.tile([VA, F1 * SC1], BF, tag="xa")
                xb = xpool.tile([VB, F1 * SC1], BF, tag="xb")
                nc.sync.dma_start(xa[:], xk[0:VA, c, :, :])
                nc.sync.dma_start(xb[:], xk[VA:V, c, :, :])
                xsT = xtp.tile([K * F1, V * SC1], BF, tag="xsT")
                nc.sync.dma_start(xsT[0:F1, :], xk0[c, :, :])
                xs_all = xsallp.tile([108, 6 * F1 * SC1], BF, tag="xsall")
                br1 = dram.tile([648, F1 * SC1], BF, tag="br1")
                for m in range(6):
                    pss = [m1ps.tile([108, 512], F32, tag="m1ps",
                                     name=f"ps1_{m}_{q}")
                           for q in range(4)]
                    for kc, (tt, xx) in enumerate(((tA, xa), (tB, xb))):
                        lw = tt[:, m * 108:(m + 1) * 108]
                        for q in range(4):
                            nc.tensor.matmul(
                                pss[q][:], lw, xx[:, q * 512:(q + 1) * 512],
                                start=(kc == 0), stop=(kc == 1))
                    base = m * F1 * SC1
                    for q in range(4):
                        nc.scalar.copy(
                            xs_all[:, base + q * 512:base + (q + 1) * 512],
                            pss[q][:])
                # bridge leg 1: SBUF (stack-row partitions) -> DRAM
                # (stack-row major), single DMA
                W1 = F1 * SC1
                nc.sync.dma_start(
                    br1[:, :].rearrange("(m u) w -> u m w", m=6, u=108),
                    xs_all[:, :].rearrange("u (m w) -> u m w", m=6, w=W1))
                # bridge leg 2: DRAM -> xsT[(k f), (u s)], one DMA per k
                brk = br1[:, :].rearrange("(k u) (f s) -> k f u s",
                                          k=4, u=V, f=F1, s=SC1)
                for k in range(1, K):
                    nc.gpsimd.dma_start(
                        xsT[k * F1:(k + 1) * F1, :], brk[k - 1])
                return xsT

            def l1_proj(c, xsT, m2ps, stscr1):
                sl = c * SC1
                # projection: psum per 4-u slot (N=512, one full bank);
                # band j on column strip j; last slot holds 2 u
                for si, (u0, du) in enumerate(L1SLOTS):
                    rows = 128 if u0 + du <= 36 else 96
                    nw = du * SC1
                    ps2 = m2ps.tile([128, 512], F32, tag="m2ps")
                    for j in range(4):
                        if u0 >= 36 and j == 3:
                            continue
                        col0 = (UB[j] + u0) * SC1
                        nc.tensor.matmul(
                            ps2[32 * j:32 * j + 32, 0:nw], w1t[:],
                            xsT[:, col0:col0 + nw],
                            start=True, stop=True,
                            tile_position=(0, 32 * j))
                    slot = c * 14 + si
                    st = stscr1[0:rows, slot * 8:slot * 8 + 6]
                    nc.vector.bn_stats(st, ps2[0:rows, 0:nw])
                    dst = ygr[0:rows, u0:u0 + du, sl:sl + SC1]
                    src = ps2[0:rows, 0:nw].rearrange(
                        "p (r s) -> p r s", r=du, s=SC1)
                    nc.scalar.copy(dst, src)

            def l2_front(c, h1p, hbp, xbp, xs2p, xta, m1ps):
                sl = c * SC2
                # JIT normalize+relu of this s-slice (all bands), in place
                nc.scalar.activation(
                    ysl[:, :, :, sl:sl + SC2], ysl[:, :, :, sl:sl + SC2],
                    mybir.ActivationFunctionType.Relu,
                    bias=par1[:, 1:2], scale=par1[:, 0:1])
                # bridge 2a via DRAM (o-major): slab -> br2a [o, (u s)]
                br2a = dram.tile([F2, V * SC2], BF, tag="br2a")
                for b in range(4):
                    w = BW[b]
                    nc.sync.dma_start(
                        br2a[:, UB[b] * SC2:(UB[b] + w) * SC2],
                        ygr[32 * b:32 * b + 32, 0:w, sl:sl + SC2])
                # h [v, (o s)] for cheb2 rhs
                ha = h1p.tile([VA, F2 * SC2], BF, tag="ha")
                hb = hbp.tile([VB, F2 * SC2], BF, tag="hb")
                bav = br2a[:, :].rearrange("o (u s) -> u o s", u=V, s=SC2)
                for j in range(3):
                    dst = ha[UB[j]:UB[j] + 42, :].rearrange(
                        "u (o s) -> u o s", o=F2, s=SC2)
                    nc.sync.dma_start(dst, bav[UB[j]:UB[j] + 42])
                dst = ha[126:128, :].rearrange("u (o s) -> u o s",
                                               o=F2, s=SC2)
                nc.sync.dma_start(dst, bav[126:128])
                dst = hb[0:VB, :].rearrange("u (o s) -> u o s", o=F2, s=SC2)
                nc.sync.dma_start(dst, bav[128:162])
                # k=0 rows of the projection rhs: straight copy of br2a
                xsTb = xbp.tile([F2, V * SC2], BF, tag="xsTb")
                nc.sync.dma_start(xsTb[:, :], br2a[:, :])
                # cheb2 on T_1..T_4
                xs2 = xs2p.tile([108, 6 * F2 * SC2], BF, tag="xs2")
                br2 = dram.tile([648, F2 * SC2], BF, tag="br2")
                W2 = F2 * SC2
                for m in range(6):
                    pss = [m1ps.tile([108, 512], F32, tag="m1ps",
                                     name=f"ps2_{m}_{q}")
                           for q in range(4)]
                    for kc, (tt, hh) in enumerate(((tA, ha), (tB, hb))):
                        lw = tt[:, m * 108:(m + 1) * 108]
                        for q in range(4):
                            nc.tensor.matmul(
                                pss[q][:], lw,
                                hh[:, q * 512:(q + 1) * 512],
                                start=(kc == 0), stop=(kc == 1))
                    base = m * W2
                    for q in range(4):
                        nc.vector.tensor_copy(
                            xs2[:, base + q * 512:base + (q + 1) * 512],
                            pss[q][:])
                nc.sync.dma_start(
                    br2[:, :].rearrange("(m u) w -> u m w", m=6, u=108),
                    xs2[:, :].rearrange("u (m w) -> u m w", m=6, w=W2))
                xsTa = xta.tile([128, V * SC2], BF, tag="xsTa")
                brk = br2[:, :].rearrange("(k u) (g s) -> k g u s",
                                          k=4, u=V, g=F2, s=SC2)
                for k in range(1, K):
                    nc.gpsimd.dma_start(
                        xsTa[(k - 1) * F2:k * F2, :], brk[k - 1])
                return xsTa, xsTb

            def l2_proj(c, xsTa, xsTb, m2ps, stscr2):
                sl = c * SC2
                # projection: psum per g; contraction (k=1..4)x32 then k=0
                for g in range(7):
                    rows = 128 if g < 6 else 96
                    ps2 = m2ps.tile([128, 384], F32, tag="m2ps")
                    for j in range(4):
                        if g == 6 and j == 3:
                            continue
                        col0 = (UB[j] + 6 * g) * SC2
                        nc.tensor.matmul(
                            ps2[32 * j:32 * j + 32, :], w2at[:],
                            xsTa[:, col0:col0 + 384],
                            start=True, stop=False,
                            tile_position=(0, 32 * j))
                        nc.tensor.matmul(
                            ps2[32 * j:32 * j + 32, :], w20t[:],
                            xsTb[:, col0:col0 + 384],
                            start=False, stop=True,
                            tile_position=(0, 32 * j))
                    slot = c * 7 + g
                    st = stscr2[0:rows, slot * 8:slot * 8 + 6]
                    nc.vector.bn_stats(st, ps2[0:rows, :])
                    dst = ysl[0:rows, g, :, sl:sl + SC2]
                    src = ps2[0:rows, :].rearrange(
                        "p (r s) -> p r s", r=6, s=SC2)
                    nc.scalar.copy(dst, src)

            def bn_finalize(stscr, gbt, par, tag):
                # per-row (mean, var) -> count-weighted (E, S) -> AllReduce ->
                # band-fold -> scale/shift
                sv = stscr[:, :].rearrange("p (n e) -> p n e", n=NSLOT, e=8)
                mv = spool.tile([128, 2], F32, tag=f"mv{tag}")
                nc.vector.bn_aggr(mv[:], sv[:, :, 0:6])
                es = spool.tile([128, 2], F32, tag=f"es{tag}")
                nc.vector.tensor_mul(es[:, 1:2], mv[:, 0:1], mv[:, 0:1])
                nc.vector.tensor_add(es[:, 1:2], es[:, 1:2], mv[:, 1:2])
                nc.vector.tensor_copy(es[:, 0:1], mv[:, 0:1])
                nc.vector.tensor_mul(es[:, 0:1], es[:, 0:1], wrt[:, 0:1])
                nc.vector.tensor_mul(es[:, 1:2], es[:, 1:2], wrt[:, 0:1])
                cin = dram.tile([128, 2], F32, tag=f"cin{tag}")
                cout = dram.tile([128, 2], F32, tag=f"cout{tag}")
                nc.gpsimd.dma_start(cin[:], es[:])
                if not SKIP_CC:
                    nc.gpsimd.collective_compute(
                        "AllReduce", mybir.AluOpType.add,
                        replica_groups=[list(range(N_CORES))],
                        ins=[cin[:].opt()], outs=[cout[:].opt()])
                else:
                    nc.gpsimd.dma_start(cout[:], cin[:])
                qs = spool.tile([32, 8], F32, tag=f"qs{tag}")
                nc.sync.dma_start(
                    qs[:].rearrange("o (j e) -> o j e", j=4, e=2),
                    cout[:].rearrange("(j o) e -> o j e", j=4, o=32))
                acc = spool.tile([32, 6], F32, tag=f"acc{tag}")
                nc.vector.tensor_add(acc[:, 0:2], qs[:, 0:2], qs[:, 2:4])
                nc.vector.tensor_add(acc[:, 2:4], qs[:, 4:6], qs[:, 6:8])
                nc.vector.tensor_add(acc[:, 0:2], acc[:, 0:2], acc[:, 2:4])
                # acc[:,0]=global mean, acc[:,1]=global E[y^2]
                nc.vector.tensor_mul(acc[:, 2:3], acc[:, 0:1], acc[:, 0:1])
                nc.vector.tensor_sub(acc[:, 1:2], acc[:, 1:2], acc[:, 2:3])
                nc.vector.tensor_scalar_add(acc[:, 1:2], acc[:, 1:2], EPS)
                nc.scalar.sqrt(acc[:, 2:3], acc[:, 1:2])
                nc.vector.reciprocal(acc[:, 3:4], acc[:, 2:3])
                nc.vector.tensor_mul(acc[:, 4:5], gbt[0:32, 0:1], acc[:, 3:4])
                nc.vector.tensor_mul(acc[:, 5:6], acc[:, 0:1], acc[:, 4:5])
                nc.vector.tensor_sub(acc[:, 5:6], gbt[0:32, 1:2], acc[:, 5:6])
                for j in range(4):
                    nc.sync.dma_start(par[32 * j:32 * j + 32, 0:1],
                                      acc[:, 4:5])
                    nc.sync.dma_start(par[32 * j:32 * j + 32, 1:2],
                                      acc[:, 5:6])

            # ---- layer 1 ----
            with (
                tc.tile_pool(name="x", bufs=2) as xpool,
                tc.tile_pool(name="xsall", bufs=1) as xsallp,
                tc.tile_pool(name="xsT", bufs=1) as xtp,
                tc.tile_pool(name="m1ps", bufs=4, space="PSUM") as m1ps,
                tc.tile_pool(name="m2ps", bufs=4, space="PSUM") as m2ps,
                tc.tile_pool(name="s1", bufs=1) as s1pool,
            ):
                stscr1 = s1pool.tile([128, STSCR_W], F32)
                nc.gpsimd.memset(stscr1[:], 0.0)
                pend = {}
                for c in range(NCH1):
                    pend[c] = l1_front(c, xpool, xsallp, xtp, m1ps)
                    if c >= 1:
                        l1_proj(c - 1, pend.pop(c - 1), m2ps, stscr1)
                l1_proj(NCH1 - 1, pend.pop(NCH1 - 1), m2ps, stscr1)
                bn_finalize(stscr1, gb1t, par1, "1")

            # ---- layer 2 ----
            with (
                tc.tile_pool(name="h1", bufs=2) as h1p,
                tc.tile_pool(name="hb", bufs=2) as hbp,
                tc.tile_pool(name="xsTb", bufs=1) as xbp,
                tc.tile_pool(name="xs2", bufs=1) as xs2p,
                tc.tile_pool(name="xsTa", bufs=2) as xta,
                tc.tile_pool(name="m1ps2", bufs=4, space="PSUM") as m1ps2,
                tc.tile_pool(name="m2ps2", bufs=4, space="PSUM") as m2ps2,
                tc.tile_pool(name="s2", bufs=1) as s2pool,
            ):
                stscr2 = s2pool.tile([128, STSCR_W], F32)
                nc.gpsimd.memset(stscr2[:], 0.0)
                pend = {}
                for c in range(NCH2):
                    pend[c] = l2_front(c, h1p, hbp, xbp, xs2p, xta, m1ps2)
                    if c >= 1:
                        l2_proj(c - 1, *pend.pop(c - 1), m2ps2, stscr2)
                l2_proj(NCH2 - 1, *pend.pop(NCH2 - 1), m2ps2, stscr2)
                bn_finalize(stscr2, gb2t, par2, "2")

            # ---- final normalize + relu + store (full-S, per band) ----
            with tc.tile_pool(name="stg", bufs=1) as stg:
                so = stg.tile([128, 42 * S], BF)
                nc.scalar.activation(
                    so[:, :], yslab[:, :],
                    mybir.ActivationFunctionType.Relu,
                    bias=par2[:, 1:2], scale=par2[:, 0:1])
                for b in range(4):
                    r0, r1 = 32 * b, 32 * b + 32
                    w = BW[b] * S
                    nc.sync.dma_start(
                        out[:, UB[b]:UB[b + 1], :], so[r0:r1, 0:w])
    nc.compile()
    return nc


def _host_prep(x, lap, w1, w2, g1, be1, g2, be2):
    lap64 = np.asarray(lap).astype(np.float64)
    T = [np.eye(V), lap64]
    for _ in range(2, K):
        T.append(2.0 * lap64 @ T[-1] - T[-2])
    tsk = np.concatenate([T[k].T for k in range(1, K)], axis=1)  # [162, 648]
    w1f = np.asarray(w1).reshape(K * F1, F2)
    w2f = np.asarray(w2).reshape(K * F2, F2)
    gb1 = np.stack([np.tile(np.asarray(g1), 4), np.tile(np.asarray(be1), 4)],
                   axis=1)
    gb2 = np.stack([np.tile(np.asarray(g2), 4), np.tile(np.asarray(be2), 4)],
                   axis=1)
    # per-row weight: n_row / total; rows 32j+o weigh band j
    nrow = np.repeat(np.array(BW, np.float64) * S, 32)
    denom = (1.0 if SKIP_CC else float(N_CORES)) * V * S
    wrow = (nrow / denom).astype(np.float32)[:, None]
    common = {
        "tsk": tsk.astype(BF16),
        "w1r": w1f.astype(BF16),
        "w2a": w2f[F2:].astype(BF16), "w20": w2f[0:F2].astype(BF16),
        "gb1": gb1.astype(np.float32), "gb2": gb2.astype(np.float32),
        "wrow": wrow,
    }
    in_maps = []
    xf = np.asarray(x).reshape(2, F1, V, 4096)
    for core in range(N_CORES):
        b, q = core // 4, core % 4
        xs = xf[b, :, :, q * S:(q + 1) * S]            # [16, 162, 1024]
        xkc = xs.transpose(1, 0, 2).reshape(V, F1, NCH1, SC1)
        xkc = xkc.transpose(0, 2, 1, 3)                # [162, 8, 16, 128]
        xk0 = xs.reshape(F1, V, NCH1, SC1)
        xk0 = xk0.transpose(2, 0, 1, 3).reshape(NCH1, F1, V * SC1)
        m = dict(common)
        m["xk"] = np.ascontiguousarray(xkc).astype(BF16)
        m["xk0"] = np.ascontiguousarray(xk0).astype(BF16)
        in_maps.append(m)
    return in_maps


_CACHE = {}


def _run(in_maps, trace=False):
    if "nc" not in _CACHE:
        _CACHE["nc"] = build_program()
    return run_bass_kernel_spmd(
        _CACHE["nc"], in_maps, core_ids=list(range(N_CORES)), trace=trace)


def kernel(x, lap, w1, b1, g1, be1, w2, b2, g2, be2, _trace=False):
    # conv biases b1/b2 cancel exactly inside BatchNorm -> ignored
    in_maps = _host_prep(x, lap, w1, w2, g1, be1, g2, be2)
    res = _run(in_maps, trace=_trace)
    _CACHE["last_results"] = res
    full = np.empty((2, F2, V, 4096), np.float32)
    for core in range(N_CORES):
        b, q = core // 4, core % 4
        full[b, :, :, q * S:(q + 1) * S] = \
            res.results[core]["out"].astype(np.float32)
    return full.reshape(2, F2, V, 16, 16, 16)
